# revision 24
# baseline (speedup 1.0000x reference)
"""Trainium2 Bass kernel for nn_Node2Pair_bias (LayerNorm -> dual projection ->
pair outer-product -> head-mix linear).

Reference computation (B=2, L=512, D=256, DH=32, H=16, K=2, P=128):
    x   = LayerNorm(node) * gamma + beta, masked        [B, L, D]
    left  = (x @ W_left + b_left)                       [B, L, DH] -> [B,L,H,K]
    right = (x @ W_right + b_right)/sqrt(DH)            [B, L, DH] -> [B,L,H,K]
    out[b,i,j,h] = sum_k left[b,i,h,k]*right[b,j,h,k]
    out[b,i,j,p] = sum_h out[b,i,j,h]*W_out[h,p] + b_out[p]   [B, L, L, P]

Restructuring (c = (h,k) channel, 0..31; W2[c,p] = W_out[c//2, p]):
    out[b,i,j,p] = sum_c right[b,j,c] * left[b,i,c] * W2[c,p]   (+ b_out)

FAST PATH (used when beta == 0, b_right == 0, b_out == 0 — always true for the
benchmark inputs).  LayerNorm is folded algebraically into the projections:
    left[c,i]  = rsm_m[i] * U_l[c,i] + b_l[c]
    right[c,j] = rsm_m[j] * U_r[c,j]
    U[c,t]     = sum_d W_e[d,c]*node[d,t] - mu[t]*s[c],   s[c] = sum_d W_e[d,c]
so the projection runs on RAW transposed node (host-provided bf16 nodeT — no
on-chip transposes), the -mu*s correction is a rank-1 outer-product matmul, and
the per-j rsm_m scale is applied FOR FREE inside the PSUM->SBUF copy
(per-partition scalar).  W_e is the gamma-folded weight; rsm_m = mask/sigma.

PE row-tiling: the pair matmul has K=32, so the 128x128 array is split into
4 row-strips (tile_position=(32q,0)) running CONCURRENTLY (~3x measured).  The
M_pack rhs is built STACKED [(q,c), (g,p)]: one DVE tensor_scalar per 4 i's
(strip q reads partitions 32q:32q+32 = its own i).  U_r is replicated 4x along
partitions by the host-replicated projection weights (free on the PE).

Each strip writes its own PSUM bank (required by row tiling); PSUM->SBUF
copies are [128, 2048] (4 banks) casting f32->bf16 with the rsm_m scale, split
between DVE and ACT.  Device output is bf16 (tolerance 2e-2; measured error
~1e-2 max), upcast on the host.  Stores are 512 KiB per (b,sp,jc,half) on
alternating HWDGE/SWDGE queues.

Sharding: sequence-parallel over the i axis of L: each core takes a [B, 64]
slice of `left` tokens plus the full `right` side, writing [B, 64, L, P] with
no cross-device communication.

The GENERAL path (nonzero beta / b_right / b_out) keeps the straightforward
structure: explicit on-chip LayerNorm + transposes, true rightT with a bias
row, plain copies.  Slower but exact for the self-test.
"""

import os
import sys

sys.path.insert(0, "/opt/trn_rl_repo")

import numpy as np

import concourse.bass as bass
import concourse.mybir as mybir
import concourse.tile as tile
from concourse import bacc
from concourse.bass_utils import run_bass_kernel_spmd
from concourse.masks import make_identity

F32 = mybir.dt.float32
BF16 = mybir.dt.bfloat16

B, L, D = 2, 512, 256
DH, H, PAIR = 32, 16, 128
NCORES = 8
LSH = L // NCORES          # 64 i's per core per batch
LN_EPS = 1e-5

_COMPILED = None           # (nc, names) of the most recently used program
_CACHE = {}                # path -> (nc, names)


# ===================================================================== fast
def _build_fast():
    nc = bacc.Bacc("TRN2", target_bir_lowering=False, debug=False,
                   num_devices=NCORES)

    def din(name, shape, dt=F32):
        return nc.dram_tensor(name, list(shape), dt, kind="ExternalInput").ap()

    # consolidated inputs: few big DMAs (per-queue issue+completion is ~1us
    # each, so 20 small loads would burn 10+us of head)
    node_cols_sh = din("node_cols_sh", (128, D), BF16)    # stats (bf16 ok)
    node_cols0 = din("node_cols0", (128, 4 * D), BF16)    # b0 tiles, [r,(t,d)]
    node_cols1 = din("node_cols1", (128, 4 * D), BF16)    # b1 tiles
    nodeT = din("nodeT", (D, B * L), BF16)                # transposed, proj
    ndTs_pack = din("ndTs_pack", (128, 256), BF16)        # shard T, [(r),(dc,i)]
    masks = din("masks", (128, 9))                        # col0 shard, 1+t full
    wlq = din("wlq", (128, 1024), BF16)                   # 8 blocks (dc,q)
    wr4_pack = din("wr4_pack", (128, 256), BF16)          # [(d%128),(dc,qc)]
    consts_row = din("consts_row", (1, 1152), BF16)       # s_r4|s_lq|ones_q
    w24bl4 = din("w24bl4", (128, 129))                    # w2_4 | b_l4

    # out[b, jc, sp, h, j, q, g, p]; i_in_core = b*64 + sp*32 + (h*4+g)*4 + q
    out = nc.dram_tensor("out", [B, 4, 2, 2, 128, 4 * 4 * PAIR], BF16,
                         kind="ExternalOutput").ap()

    with tile.TileContext(nc) as tc:
        with (
            tc.tile_pool(name="singles", bufs=1) as singles,
            tc.tile_pool(name="xrows", bufs=3) as xrows,
            tc.tile_pool(name="stats", bufs=4) as stats,
            tc.tile_pool(name="persist", bufs=1) as persist,
            tc.tile_pool(name="mp", bufs=3) as mp_pool,
            tc.tile_pool(name="stag", bufs=12) as stag_pool,
            tc.tile_pool(name="ps", bufs=4, space="PSUM") as ps,
        ):
            ident = singles.tile([128, 128], F32, tag="ident")
            make_identity(nc, ident)
            eps_t = singles.tile([128, 1], F32, tag="eps")
            nc.vector.memset(eps_t, LN_EPS)

            # ---------- loads ----------
            xfb = [xrows.tile([128, 4 * D], BF16, tag="xf", name=f"xfb{b}")
                   for b in range(B)]
            nc.sync.dma_start(out=xfb[0][:, 0:2 * D],
                              in_=node_cols0[:, 0:2 * D])
            msk = singles.tile([128, 9], F32, tag="msk")
            nc.sync.dma_start(out=msk, in_=masks[:, :])
            # tiles 2-3 on the gpsimd queue so their stat chains don't wait
            # behind the sync queue's ndT loads (head-of-line on DVE)
            nc.gpsimd.dma_start(out=xfb[0][:, 2 * D:4 * D],
                                in_=node_cols0[:, 2 * D:4 * D])
            ndT = [[persist.tile([128, L], BF16, tag=f"ndT{b}_{dc}",
                                 name=f"ndT{b}_{dc}") for dc in range(2)]
                   for b in range(B)]
            for dc in range(2):
                nc.sync.dma_start(out=ndT[0][dc],
                                  in_=nodeT[dc * 128:(dc + 1) * 128, 0:L])
            ndTs = singles.tile([128, 256], BF16, tag="ndTs")
            nc.sync.dma_start(out=ndTs, in_=ndTs_pack[:, :])

            xsr = xrows.tile([128, D], BF16, tag="xs", name="xsr")
            nc.scalar.dma_start(out=xsr, in_=node_cols_sh[:, :])
            wlq_sb = singles.tile([128, 1024], BF16, tag="wlq")
            nc.scalar.dma_start(out=wlq_sb, in_=wlq[:, :])
            crow = singles.tile([1, 1152], BF16, tag="crow")
            nc.scalar.dma_start(out=crow, in_=consts_row[:, :])
            w24bl = singles.tile([128, 129], F32, tag="w24bl")
            nc.scalar.dma_start(out=w24bl, in_=w24bl4[:, :])
            wr4_sb = singles.tile([128, 256], BF16, tag="wr4")
            nc.scalar.dma_start(out=wr4_sb, in_=wr4_pack[:, :])

            nc.gpsimd.dma_start(out=xfb[1], in_=node_cols1[:, :])
            for dc in range(2):
                nc.gpsimd.dma_start(out=ndT[1][dc],
                                    in_=nodeT[dc * 128:(dc + 1) * 128, L:2 * L])

            sr4_sb = crow[0:1, 0:128]
            slq_sb = crow[0:1, 128:640]
            onq_sb = crow[0:1, 640:1152]
            w24_sb = w24bl[:, 0:128]
            bl4_sb = w24bl[:, 128:129]

            # ---------- stat chain helper ----------
            def stat_chain(x_ap, mask_ap, rsm_dst):
                st = stats.tile([128, 6], F32, tag="st")
                nc.vector.bn_stats(out=st, in_=x_ap)
                mv = stats.tile([128, 2], F32, tag="mv")
                nc.vector.bn_aggr(out=mv, in_=st)
                sd = stats.tile([128, 1], F32, tag="sd")
                nc.scalar.activation(out=sd, in_=mv[:, 1:2],
                                     func=mybir.ActivationFunctionType.Sqrt,
                                     bias=eps_t, scale=1.0)
                rs = stats.tile([128, 1], F32, tag="rs")
                nc.vector.reciprocal(out=rs, in_=sd)
                nc.scalar.activation(out=rsm_dst, in_=rs,
                                     func=mybir.ActivationFunctionType.Copy,
                                     scale=mask_ap)
                return mv

            # ---------- shard path -> leftT_st [(q,c), 32 groups] ----------
            # single PSUM slot: psU [:,0:32] | psBc [:,256:288] | rows 512:768
            rsm_sh = singles.tile([128, 1], F32, tag="rsmsh")
            negmu_sh = singles.tile([1, 128], BF16, tag="negmush")
            rsmrow_sh = singles.tile([1, 128], BF16, tag="rsmrowsh")
            U_lst = singles.tile([128, 32], F32, tag="ulst")
            leftT_st = persist.tile([128, 32], F32, tag="leftst")
            sh_state = {}

            def shard_stats():
                mv_sh = stat_chain(xsr, msk[:, 0:1], rsm_sh)
                psh = ps.tile([128, 1024], F32, tag="ps", name="psh")
                nc.tensor.transpose(psh[0:1, 512:640], mv_sh[:, 0:1], ident)
                nc.tensor.transpose(psh[0:1, 640:768], rsm_sh, ident)
                nc.scalar.activation(out=negmu_sh, in_=psh[0:1, 512:640],
                                     func=mybir.ActivationFunctionType.Copy,
                                     scale=-1.0)
                nc.scalar.copy(out=rsmrow_sh, in_=psh[0:1, 640:768])
                sh_state["psh"] = psh

            def strided4(ap_row, q):
                r = ap_row.rearrange("r (g q) -> r q g", q=4)
                return r[:, q:q + 1, :]

            def build_left():
                psh = sh_state["psh"]
                psU = psh[:, 0:32]
                for dc in range(2):
                    nd3 = ndTs[:, dc * 128:(dc + 1) * 128].rearrange(
                        "d (g q) -> d q g", q=4)
                    for q in range(4):
                        nc.tensor.matmul(psU,
                                         wlq_sb[:, (dc * 4 + q) * 128:
                                                (dc * 4 + q + 1) * 128],
                                         nd3[:, q:q + 1, :],
                                         start=(dc == 0 and q == 0),
                                         stop=False)
                for q in range(4):
                    nc.tensor.matmul(psU, slq_sb[0:1, q * 128:(q + 1) * 128],
                                     strided4(negmu_sh, q),
                                     start=False, stop=(q == 3))
                nc.vector.tensor_copy(out=U_lst, in_=psU)
                psBc = psh[:, 256:288]
                for q in range(4):
                    nc.tensor.matmul(psBc, onq_sb[0:1, q * 128:(q + 1) * 128],
                                     strided4(rsmrow_sh, q),
                                     start=(q == 0), stop=(q == 3))
                nc.vector.tensor_mul(out=leftT_st, in0=U_lst, in1=psBc)
                nc.vector.tensor_scalar_add(out=leftT_st, in0=leftT_st,
                                            scalar1=bl4_sb)

            # ---------- right path per b -> U_r4[b] [(q,c), 512] ----------
            rsmc = persist.tile([128, 8], F32, tag="rsmc")   # col t = b*4+jc
            U_r4 = [persist.tile([128, L], BF16, tag=f"ur4{b}", name=f"ur4{b}")
                    for b in range(B)]

            br_state = {}

            def build_right(b, ts=(0, 1, 2, 3)):
                # per-jc slices: the first store depends on ONE stat chain,
                # not all four.  single PSUM slot: psUr [:,0:512] | mu rows
                if b not in br_state:
                    br_state[b] = (
                        ps.tile([128, 1024], F32, tag="ps", name=f"pbr{b}"),
                        singles.tile([1, 512], BF16, tag=f"negmu{b}",
                                     name=f"negmu{b}"))
                pb_t, negmu = br_state[b]
                psUr = pb_t[:, 0:512]
                for t in ts:
                    tt = b * 4 + t
                    mv = stat_chain(xfb[b][:, t * D:(t + 1) * D],
                                    msk[:, 1 + tt:2 + tt], rsmc[:, tt:tt + 1])
                    tsl = slice(t * 128, (t + 1) * 128)
                    nc.tensor.transpose(pb_t[0:1, 512 + t * 128:
                                             512 + (t + 1) * 128],
                                        mv[:, 0:1], ident)
                    nc.scalar.activation(out=negmu[0:1, tsl],
                                         in_=pb_t[0:1, 512 + t * 128:
                                                  512 + (t + 1) * 128],
                                         func=mybir.ActivationFunctionType.Copy,
                                         scale=-1.0)
                    nc.tensor.matmul(psUr[:, tsl], sr4_sb, negmu[0:1, tsl],
                                     start=True, stop=False)
                    nc.tensor.matmul(psUr[:, tsl], wr4_sb[:, 0:128],
                                     ndT[b][0][:, tsl],
                                     start=False, stop=False)
                    nc.tensor.matmul(psUr[:, tsl], wr4_sb[:, 128:256],
                                     ndT[b][1][:, tsl],
                                     start=False, stop=True)
                    if t % 2 == 0:
                        nc.vector.tensor_copy(out=U_r4[b][:, tsl],
                                              in_=psUr[:, tsl])
                    else:
                        nc.scalar.copy(out=U_r4[b][:, tsl], in_=psUr[:, tsl])

            # ---------- main loop ----------
            copy_cnt = [0]

            def scaled_copy(dst, src, scale_ap):
                on_dve = (copy_cnt[0] * 7) % 16 < 7
                copy_cnt[0] += 1
                if on_dve:
                    nc.vector.tensor_scalar_mul(out=dst, in0=src,
                                                scalar1=scale_ap)
                else:
                    nc.scalar.activation(out=dst, in_=src,
                                         func=mybir.ActivationFunctionType.Copy,
                                         scale=scale_ap)

            def main_block(b, sp):
                mp_all = mp_pool.tile([128, 1024], BF16, tag="mp",
                                      name=f"mp{b}_{sp}")
                for g in range(8):
                    col = b * 16 + sp * 8 + g
                    osl = mp_all[:, g * 128:(g + 1) * 128]
                    if b == 0 and sp == 0 and g % 2 == 1:
                        # first block: build on both engines to cut the head
                        nc.scalar.activation(
                            out=osl, in_=w24_sb,
                            func=mybir.ActivationFunctionType.Copy,
                            scale=leftT_st[:, col:col + 1])
                    else:
                        nc.vector.tensor_scalar_mul(
                            out=osl, in0=w24_sb,
                            scalar1=leftT_st[:, col:col + 1])
                for jc in range(4):
                    tt = b * 4 + jc
                    stg = stag_pool.tile([128, 4096], BF16, tag="stag",
                                         name=f"stg{b}{sp}{jc}")
                    for h in range(2):
                        pt = [ps.tile([128, 1024], F32, tag="ps",
                                      name=f"pt{b}{sp}{jc}{h}{k}")
                              for k in range(2)]
                        for q in range(4):
                            nc.tensor.matmul(
                                pt[q // 2][:, (q % 2) * 512:
                                           (q % 2 + 1) * 512],
                                U_r4[b][32 * q:32 * q + 32,
                                        jc * 128:(jc + 1) * 128],
                                mp_all[32 * q:32 * q + 32,
                                       h * 512:(h + 1) * 512],
                                start=True, stop=True,
                                tile_position=(32 * q, 0))
                        for k in range(2):
                            scaled_copy(stg[:, h * 2048 + k * 1024:
                                            h * 2048 + (k + 1) * 1024],
                                        pt[k], rsmc[:, tt:tt + 1])
                        nc.sync.dma_start(
                            out=out[b, jc, sp, h, :, :],
                            in_=stg[:, h * 2048:(h + 1) * 2048])

            shard_stats()
            build_right(0, (0, 1))
            build_left()
            build_right(0, (2, 3))
            main_block(0, 0)
            build_right(1)
            main_block(0, 1)
            main_block(1, 0)
            main_block(1, 1)

    nc.compile()
    names = ["node_cols_sh", "node_cols0", "node_cols1", "nodeT", "ndTs_pack",
             "masks", "wlq", "wr4_pack", "consts_row", "w24bl4"]
    return nc, names


def _prep_fast(node, mask, ln_gamma, ln_beta, W_left, b_left, W_right,
               b_right, W_out, b_out):
    import ml_dtypes
    f = np.float32
    bf = ml_dtypes.bfloat16
    node = np.ascontiguousarray(np.asarray(node, dtype=f))        # [B, L, D]
    mask_f = np.asarray(mask).astype(f)
    gamma = np.asarray(ln_gamma, dtype=f)
    W_l = np.asarray(W_left, dtype=f)
    W_r = np.asarray(W_right, dtype=f)
    b_l = np.asarray(b_left, dtype=f)
    W_o = np.asarray(W_out, dtype=f)

    s = 1.0 / np.sqrt(np.float32(DH))
    W_le = gamma[:, None] * W_l                    # [256, 32]
    W_re = gamma[:, None] * W_r * s
    s_l = W_le.sum(0)
    s_r = W_re.sum(0)

    wlq = np.zeros((8, 128, 128), f)
    for dc in range(2):
        for q in range(4):
            wlq[dc * 4 + q, :, 32 * q:32 * q + 32] = \
                W_le[dc * 128:(dc + 1) * 128, :]
    crow = np.zeros((1, 1152), f)
    crow[0, 0:128] = np.tile(s_r, 4)
    for q in range(4):
        crow[0, 128 + q * 128 + 32 * q:128 + q * 128 + 32 * q + 32] = s_l
        crow[0, 640 + q * 128 + 32 * q:640 + q * 128 + 32 * q + 32] = 1.0

    node_flat = node.reshape(B * L, D)
    mcol = np.zeros((128, 9), f)
    mcol[:, 1:9] = mask_f.reshape(-1, 128).T
    w24bl4 = np.empty((128, 129), f)
    w24bl4[:, 0:128] = np.tile(np.repeat(W_o, 2, axis=0), (4, 1))
    w24bl4[:, 128] = np.tile(b_l, 4)
    common = {
        "node_cols0": np.ascontiguousarray(
            node_flat[0:512].reshape(4, 128, D).transpose(1, 0, 2)
            .reshape(128, 4 * D)).astype(bf),
        "node_cols1": np.ascontiguousarray(
            node_flat[512:1024].reshape(4, 128, D).transpose(1, 0, 2)
            .reshape(128, 4 * D)).astype(bf),
        "nodeT": np.ascontiguousarray(node_flat.T).astype(bf),
        "wlq": np.ascontiguousarray(
            wlq.transpose(1, 0, 2).reshape(128, 1024)).astype(bf),
        "wr4_pack": np.ascontiguousarray(
            np.tile(W_re, (1, 4)).reshape(2, 128, 128).transpose(1, 0, 2)
            .reshape(128, 256)).astype(bf),
        "consts_row": crow.astype(bf),
        "w24bl4": w24bl4,
    }

    in_maps = []
    for c in range(NCORES):
        sl = slice(c * LSH, (c + 1) * LSH)
        shard = np.ascontiguousarray(node[:, sl, :].reshape(B * LSH, D))
        msk = mask_f[:, sl]
        m = dict(common)
        m["node_cols_sh"] = shard.astype(bf)
        m["ndTs_pack"] = np.ascontiguousarray(
            shard.T.reshape(2, 128, 128).transpose(1, 0, 2)
            .reshape(128, 256)).astype(bf)
        mc = mcol.copy()
        mc[:, 0] = msk.reshape(-1)
        m["masks"] = mc
        in_maps.append(m)
    return in_maps


def _unpack_fast(res):
    full = np.empty((B, L, L, PAIR), np.float32)
    for c in range(NCORES):
        dev = np.asarray(res.results[c]["out"]).reshape(
            B, 4, 2, 2, 128, 4, 4, PAIR)             # [b,jc,sp,h,j,q,g,p]
        full[:, c * LSH:(c + 1) * LSH] = (
            dev.transpose(0, 2, 3, 6, 5, 1, 4, 7)      # b,sp,h,g,q,jc,j,p
               .reshape(B, LSH, L, PAIR).astype(np.float32))
    return full


# ================================================================== general
def _build_general():
    nc = bacc.Bacc("TRN2", target_bir_lowering=False, debug=False,
                   num_devices=NCORES)

    def din(name, shape):
        return nc.dram_tensor(name, list(shape), F32, kind="ExternalInput").ap()

    node_full = din("node_full", (B * L, D))
    node_shard = din("node_shard", (B * LSH, D))
    mask_col_full = din("mask_col_full", (128, B * L // 128))
    mask_col_shard = din("mask_col_shard", (128, 1))
    mask_row_full = din("mask_row_full", (B, L))
    mask_row_shard = din("mask_row_shard", (1, B * LSH))
    w_left_e = din("w_left_e", (D + 1, DH))
    w_right_e = din("w_right_e", (D + 1, DH))
    b_left_col = din("b_left_col", (DH, 1))
    b_right_col = din("b_right_col", (DH, 1))
    w2 = din("w2", (DH + 1, PAIR))

    out = nc.dram_tensor("out", [B, 4, 4, 128, 16, PAIR], BF16,
                         kind="ExternalOutput").ap()

    NT_FULL = B * L // 128

    with tile.TileContext(nc) as tc:
        with (
            tc.tile_pool(name="singles", bufs=1) as singles,
            tc.tile_pool(name="xpool", bufs=9) as xpool,
            tc.tile_pool(name="stats", bufs=4) as stats,
            tc.tile_pool(name="persist", bufs=1) as persist,
            tc.tile_pool(name="mp", bufs=8) as mp_pool,
            tc.tile_pool(name="stag", bufs=12) as stag_pool,
            tc.tile_pool(name="ps_tp", bufs=1, space="PSUM") as ps_tp,
            tc.tile_pool(name="ps_proj", bufs=1, space="PSUM") as ps_proj,
            tc.tile_pool(name="ps_big", bufs=5, space="PSUM") as ps_big,
        ):
            ident = singles.tile([128, 128], F32, tag="ident")
            make_identity(nc, ident)
            eps_t = singles.tile([128, 1], F32, tag="eps")
            nc.vector.memset(eps_t, LN_EPS)

            xs = xpool.tile([128, D], F32, tag="x", name="xs")
            nc.sync.dma_start(out=xs, in_=node_shard[:, :])
            mcs_sb = singles.tile([128, 1], F32, tag="mcs")
            nc.sync.dma_start(out=mcs_sb, in_=mask_col_shard[:, :])
            xf_tiles = [None] * NT_FULL
            for t in range(NT_FULL):
                xf = xpool.tile([128, D], F32, tag="x", name=f"xf{t}")
                nc.sync.dma_start(out=xf,
                                  in_=node_full[t * 128:(t + 1) * 128, :])
                xf_tiles[t] = xf
            mcf_sb = singles.tile([128, NT_FULL], F32, tag="mcf")
            nc.sync.dma_start(out=mcf_sb, in_=mask_col_full[:, :])
            wl_sb = [singles.tile([128, DH], F32, tag=f"wl{dc}", name=f"wl{dc}")
                     for dc in range(2)]
            wl_row = singles.tile([1, DH], F32, tag="wlrow")
            for dc in range(2):
                nc.sync.dma_start(out=wl_sb[dc],
                                  in_=w_left_e[dc * 128:(dc + 1) * 128, :])
            nc.sync.dma_start(out=wl_row, in_=w_left_e[D:D + 1, :])
            bl_sb = singles.tile([DH, 1], F32, tag="bl")
            nc.sync.dma_start(out=bl_sb, in_=b_left_col[:, :])
            w2_sb = singles.tile([DH + 1, PAIR], F32, tag="w2")
            nc.sync.dma_start(out=w2_sb, in_=w2[:, :])

            wr_sb = [singles.tile([128, DH], F32, tag=f"wr{dc}", name=f"wr{dc}")
                     for dc in range(2)]
            wr_row = singles.tile([1, DH], F32, tag="wrrow")
            for dc in range(2):
                nc.gpsimd.dma_start(out=wr_sb[dc],
                                    in_=w_right_e[dc * 128:(dc + 1) * 128, :])
            nc.gpsimd.dma_start(out=wr_row, in_=w_right_e[D:D + 1, :])
            br_sb = singles.tile([DH, 1], F32, tag="br")
            nc.gpsimd.dma_start(out=br_sb, in_=b_right_col[:, :])
            mrf_sb = [singles.tile([1, L], F32, tag=f"mrf{b}", name=f"mrf{b}")
                      for b in range(B)]
            for b in range(B):
                nc.gpsimd.dma_start(out=mrf_sb[b],
                                    in_=mask_row_full[b:b + 1, :])
            mrs_sb = singles.tile([1, B * LSH], F32, tag="mrs")
            nc.gpsimd.dma_start(out=mrs_sb, in_=mask_row_shard[:, :])
            ones_row = singles.tile([1, L], BF16, tag="ones")
            nc.vector.memset(ones_row, 1.0)

            def layernorm_masked(x_t, mask_col_ap):
                st = stats.tile([128, 6], F32, tag="st")
                nc.vector.bn_stats(out=st, in_=x_t)
                mv = stats.tile([128, 2], F32, tag="mv")
                nc.vector.bn_aggr(out=mv, in_=st)
                sd = stats.tile([128, 1], F32, tag="sd")
                nc.scalar.activation(out=sd, in_=mv[:, 1:2],
                                     func=mybir.ActivationFunctionType.Sqrt,
                                     bias=eps_t, scale=1.0)
                rs = stats.tile([128, 1], F32, tag="rs")
                nc.vector.reciprocal(out=rs, in_=sd)
                rsm = stats.tile([128, 1], F32, tag="rsm")
                nc.vector.tensor_mul(out=rsm, in0=rs, in1=mask_col_ap)
                nc.vector.tensor_scalar(out=x_t, in0=x_t,
                                        scalar1=mv[:, 0:1], scalar2=rsm,
                                        op0=mybir.AluOpType.subtract,
                                        op1=mybir.AluOpType.mult)

            layernorm_masked(xs, mcs_sb[:, 0:1])
            xsT = [persist.tile([128, B * LSH], F32, tag=f"xsT{dc}",
                                name=f"xsT{dc}") for dc in range(2)]
            for dc in range(2):
                pt = ps_tp.tile([128, 128], F32, tag="tp")
                nc.tensor.transpose(pt, xs[:, dc * 128:(dc + 1) * 128], ident)
                nc.scalar.copy(out=xsT[dc], in_=pt)

            ps_l = ps_proj.tile([DH, L], F32, tag="pr", name="ps_l")
            ps_l = ps_l[:, 0:B * LSH]
            for dc in range(2):
                nc.tensor.matmul(ps_l, wl_sb[dc], xsT[dc],
                                 start=(dc == 0), stop=False)
            nc.tensor.matmul(ps_l, wl_row, mrs_sb, start=False, stop=True)
            leftT = persist.tile([DH + 1, B * LSH], F32, tag="leftT")
            nc.vector.tensor_scalar_add(out=leftT[0:DH, :], in0=ps_l,
                                        scalar1=bl_sb)
            nc.vector.memset(leftT[DH:DH + 1, :], 1.0)

            rightT = [persist.tile([DH + 1, L], BF16, tag=f"rt{b}",
                                   name=f"rt{b}") for b in range(B)]
            xT = [[persist.tile([128, L], F32, tag=f"xT{b}_{dc}",
                                name=f"xT{b}_{dc}") for dc in range(2)]
                  for b in range(B)]
            for b in range(B):
                for lc in range(4):
                    t = b * 4 + lc
                    xft = xf_tiles[t]
                    layernorm_masked(xft, mcf_sb[:, t:t + 1])
                    for dc in range(2):
                        pt = ps_tp.tile([128, 128], F32, tag="tp")
                        nc.tensor.transpose(pt, xft[:, dc * 128:(dc + 1) * 128],
                                            ident)
                        if (lc + dc) % 2 == 0:
                            nc.vector.tensor_copy(
                                out=xT[b][dc][:, lc * 128:(lc + 1) * 128],
                                in_=pt)
                        else:
                            nc.scalar.copy(
                                out=xT[b][dc][:, lc * 128:(lc + 1) * 128],
                                in_=pt)

                ps_r = ps_proj.tile([DH, L], F32, tag="pr")
                for jc in range(4):
                    jsl = slice(jc * 128, (jc + 1) * 128)
                    for dc in range(2):
                        nc.tensor.matmul(ps_r[:, jsl], wr_sb[dc],
                                         xT[b][dc][:, jsl],
                                         start=(dc == 0), stop=False)
                    nc.tensor.matmul(ps_r[:, jsl], wr_row, mrf_sb[b][:, jsl],
                                     start=False, stop=True)
                    nc.vector.tensor_scalar_add(out=rightT[b][0:DH, jsl],
                                                in0=ps_r[:, jsl],
                                                scalar1=br_sb)
                nc.vector.tensor_copy(out=rightT[b][DH:DH + 1, :],
                                      in_=ones_row)

            copy_cnt = [0]

            def psum_copy(dst, src):
                on_dve = (copy_cnt[0] * 11) % 32 < 11
                copy_cnt[0] += 1
                if on_dve:
                    nc.vector.tensor_copy(out=dst, in_=src)
                else:
                    nc.scalar.copy(out=dst, in_=src)

            def build_mps(b, sg):
                mps = []
                for il in range(4):
                    mp = mp_pool.tile([DH + 1, 512], BF16, tag="mp",
                                      name=f"mp{b}_{sg}_{il}")
                    for q in range(4):
                        col = b * LSH + (sg * 4 + il) * 4 + q
                        nc.vector.tensor_scalar_mul(
                            out=mp[:, q * 128:(q + 1) * 128], in0=w2_sb,
                            scalar1=leftT[:, col:col + 1])
                    mps.append(mp)
                return mps

            for b in range(B):
                for sg in range(4):
                    mps = build_mps(b, sg)
                    for jc in range(4):
                        lhsT = rightT[b][:, jc * 128:(jc + 1) * 128]
                        stg = stag_pool.tile([128, 16 * 128], BF16, tag="stag")
                        for il in range(4):
                            pb = ps_big.tile([128, 512], F32, tag="big")
                            nc.tensor.matmul(pb, lhsT, mps[il],
                                             start=True, stop=True)
                            psum_copy(stg[:, il * 512:(il + 1) * 512], pb)
                        dst_ap = out[b, jc, sg, :, :, :]
                        src_ap = stg[:, :].rearrange("j (i p) -> j i p", p=128)
                        deng = nc.sync if jc % 2 == 0 else nc.gpsimd
                        deng.dma_start(out=dst_ap, in_=src_ap)

    nc.compile()
    names = ["node_full", "node_shard", "mask_col_full", "mask_col_shard",
             "mask_row_full", "mask_row_shard", "w_left_e", "w_right_e",
             "b_left_col", "b_right_col", "w2"]
    return nc, names


def _prep_general(node, mask, ln_gamma, ln_beta, W_left, b_left, W_right,
                  b_right, W_out, b_out):
    f = np.float32
    node = np.ascontiguousarray(np.asarray(node, dtype=f))
    mask_f = np.asarray(mask).astype(f)
    gamma = np.asarray(ln_gamma, dtype=f)
    beta = np.asarray(ln_beta, dtype=f)
    W_l = np.asarray(W_left, dtype=f)
    W_r = np.asarray(W_right, dtype=f)
    b_l = np.asarray(b_left, dtype=f)
    b_r = np.asarray(b_right, dtype=f)
    W_o = np.asarray(W_out, dtype=f)
    b_o = np.asarray(b_out, dtype=f)

    s = 1.0 / np.sqrt(np.float32(DH))
    w_left_e = np.concatenate([gamma[:, None] * W_l, (beta @ W_l)[None, :]], 0)
    w_right_e = np.concatenate([gamma[:, None] * W_r, (beta @ W_r)[None, :]],
                               0) * s
    w2 = np.concatenate([np.repeat(W_o, 2, axis=0), b_o[None, :]], 0)

    node_flat = node.reshape(B * L, D)
    common = {
        "node_full": node_flat,
        "mask_col_full": np.ascontiguousarray(mask_f.reshape(-1, 128).T),
        "mask_row_full": np.ascontiguousarray(mask_f),
        "w_left_e": np.ascontiguousarray(w_left_e),
        "w_right_e": np.ascontiguousarray(w_right_e),
        "b_left_col": np.ascontiguousarray(b_l[:, None]),
        "b_right_col": np.ascontiguousarray(b_r[:, None] * s),
        "w2": np.ascontiguousarray(w2),
    }

    in_maps = []
    for c in range(NCORES):
        sl = slice(c * LSH, (c + 1) * LSH)
        shard = np.ascontiguousarray(node[:, sl, :].reshape(B * LSH, D))
        msk = mask_f[:, sl]
        m = dict(common)
        m["node_shard"] = shard
        m["mask_col_shard"] = np.ascontiguousarray(msk.reshape(-1)[:, None])
        m["mask_row_shard"] = np.ascontiguousarray(msk.reshape(1, -1))
        in_maps.append(m)
    return in_maps


def _unpack_general(res):
    full = np.empty((B, L, L, PAIR), np.float32)
    for c in range(NCORES):
        dev = np.asarray(res.results[c]["out"])  # [b, jc, sg, j, i16, p]
        full[:, c * LSH:(c + 1) * LSH] = (
            dev.transpose(0, 2, 4, 1, 3, 5)
               .reshape(B, LSH, L, PAIR).astype(np.float32))
    return full


# ================================================================ dispatch
def _is_fast(inputs):
    z = lambda k: not np.any(np.asarray(inputs[k]))
    return z("ln_beta") and z("b_right") and z("b_out")


def _get_program(fast):
    global _COMPILED
    key = "fast" if fast else "general"
    if key not in _CACHE:
        _CACHE[key] = _build_fast() if fast else _build_general()
    _COMPILED = _CACHE[key]
    return _CACHE[key]


def _prepare_in_maps(**inputs):
    fast = _is_fast(inputs)
    _get_program(fast)
    return (_prep_fast if fast else _prep_general)(**inputs)


def kernel(**inputs):
    fast = _is_fast(inputs)
    nc, names = _get_program(fast)
    in_maps = (_prep_fast if fast else _prep_general)(**inputs)
    res = run_bass_kernel_spmd(nc, in_maps, core_ids=list(range(NCORES)))
    return (_unpack_fast if fast else _unpack_general)(res)


if __name__ == "__main__":
    rng = np.random.default_rng(1)

    def np_reference(node, mask, ln_gamma, ln_beta, W_left, b_left, W_right,
                     b_right, W_out, b_out):
        node = node.astype(np.float64)
        mu = node.mean(-1, keepdims=True)
        var = ((node - mu) ** 2).mean(-1, keepdims=True)
        x = (node - mu) / np.sqrt(var + LN_EPS) * ln_gamma + ln_beta
        x = x * mask[..., None]
        left = (x @ W_left + b_left).reshape(B, L, H, -1)
        right = ((x @ W_right + b_right) / np.sqrt(DH)).reshape(B, L, H, -1)
        o = np.einsum("bihk,bjhk->bijh", left, right)
        return np.einsum("bijh,hp->bijp", o, W_out) + b_out

    # --- fast path (benchmark-like: beta/b_r/b_out zero, mask+gamma general)
    mask = np.ones((B, L), dtype=bool)
    mask[0, 500:] = False
    mask[1, :3] = False
    fast_inputs = {
        "node": rng.standard_normal((B, L, D)).astype(np.float32),
        "mask": mask,
        "ln_gamma": (1.0 + 0.1 * rng.standard_normal(D)).astype(np.float32),
        "ln_beta": np.zeros(D, np.float32),
        "W_left": (rng.standard_normal((D, DH)) / np.sqrt(D)).astype(np.float32),
        "b_left": (0.1 * rng.standard_normal(DH)).astype(np.float32),
        "W_right": (rng.standard_normal((D, DH)) / np.sqrt(D)).astype(np.float32),
        "b_right": np.zeros(DH, np.float32),
        "W_out": (rng.standard_normal((H, PAIR)) / np.sqrt(H)).astype(np.float32),
        "b_out": np.zeros(PAIR, np.float32),
    }
    got = kernel(**fast_inputs)
    exp = np_reference(**fast_inputs)
    rel = np.abs(got - exp).max() / np.abs(exp).max()
    print("fast-path rel err:", rel)
    assert rel < 1.8e-2, rel

    # --- general path (everything nonzero)
    gen_inputs = dict(fast_inputs)
    gen_inputs["ln_beta"] = (0.1 * rng.standard_normal(D)).astype(np.float32)
    gen_inputs["b_right"] = (0.1 * rng.standard_normal(DH)).astype(np.float32)
    gen_inputs["b_out"] = (0.1 * rng.standard_normal(PAIR)).astype(np.float32)
    got = kernel(**gen_inputs)
    exp = np_reference(**gen_inputs)
    rel = np.abs(got - exp).max() / np.abs(exp).max()
    print("general-path rel err:", rel)
    assert rel < 1.8e-2, rel
    print("OK")


# revision 25
# speedup vs baseline: 1.0622x; 1.0622x over previous
"""Trainium2 Bass kernel for nn_Node2Pair_bias (LayerNorm -> dual projection ->
pair outer-product -> head-mix linear).

Reference computation (B=2, L=512, D=256, DH=32, H=16, K=2, P=128):
    x   = LayerNorm(node) * gamma + beta, masked        [B, L, D]
    left  = (x @ W_left + b_left)                       [B, L, DH] -> [B,L,H,K]
    right = (x @ W_right + b_right)/sqrt(DH)            [B, L, DH] -> [B,L,H,K]
    out[b,i,j,h] = sum_k left[b,i,h,k]*right[b,j,h,k]
    out[b,i,j,p] = sum_h out[b,i,j,h]*W_out[h,p] + b_out[p]   [B, L, L, P]

Restructuring (c = (h,k) channel, 0..31; W2[c,p] = W_out[c//2, p]):
    out[b,i,j,p] = sum_c right[b,j,c] * left[b,i,c] * W2[c,p]   (+ b_out)

FAST PATH (used when beta == 0, b_right == 0, b_out == 0 — always true for the
benchmark inputs).  LayerNorm is folded algebraically into the projections:
    left[c,i]  = rsm_m[i] * U_l[c,i] + b_l[c]
    right[c,j] = rsm_m[j] * U_r[c,j]
    U[c,t]     = sum_d W_e[d,c]*node[d,t] - mu[t]*s[c],   s[c] = sum_d W_e[d,c]
so the projection runs on RAW transposed node (host-provided bf16 nodeT — no
on-chip transposes), the -mu*s correction is a rank-1 outer-product matmul, and
the per-j rsm_m scale is applied FOR FREE inside the PSUM->SBUF copy
(per-partition scalar).  W_e is the gamma-folded weight; rsm_m = mask/sigma.

PE row-tiling: the pair matmul has K=32, so the 128x128 array is split into
4 row-strips (tile_position=(32q,0)) running CONCURRENTLY (~3x measured).  The
M_pack rhs is built STACKED [(q,c), (g,p)]: one DVE tensor_scalar per 4 i's
(strip q reads partitions 32q:32q+32 = its own i).  U_r is replicated 4x along
partitions by the host-replicated projection weights (free on the PE).

Each strip writes its own PSUM bank (required by row tiling); PSUM->SBUF
copies are [128, 2048] (4 banks) casting f32->bf16 with the rsm_m scale, split
between DVE and ACT.  Device output is bf16 (tolerance 2e-2; measured error
~1e-2 max), upcast on the host.  Stores are 512 KiB per (b,sp,jc,half) on
alternating HWDGE/SWDGE queues.

Sharding: sequence-parallel over the i axis of L: each core takes a [B, 64]
slice of `left` tokens plus the full `right` side, writing [B, 64, L, P] with
no cross-device communication.

The GENERAL path (nonzero beta / b_right / b_out) keeps the straightforward
structure: explicit on-chip LayerNorm + transposes, true rightT with a bias
row, plain copies.  Slower but exact for the self-test.
"""

import os
import sys

sys.path.insert(0, "/opt/trn_rl_repo")

import numpy as np

import concourse.bass as bass
import concourse.mybir as mybir
import concourse.tile as tile
from concourse import bacc
from concourse.bass_utils import run_bass_kernel_spmd
from concourse.masks import make_identity

F32 = mybir.dt.float32
BF16 = mybir.dt.bfloat16

B, L, D = 2, 512, 256
DH, H, PAIR = 32, 16, 128
NCORES = 8
LSH = L // NCORES          # 64 i's per core per batch
LN_EPS = 1e-5

_COMPILED = None           # (nc, names) of the most recently used program
_CACHE = {}                # path -> (nc, names)


# ===================================================================== fast
def _build_fast():
    nc = bacc.Bacc("TRN2", target_bir_lowering=False, debug=False,
                   num_devices=NCORES)

    def din(name, shape, dt=F32):
        return nc.dram_tensor(name, list(shape), dt, kind="ExternalInput").ap()

    # consolidated inputs: few big DMAs (per-queue issue+completion is ~1us
    # each, so 20 small loads would burn 10+us of head)
    node_cols_sh = din("node_cols_sh", (128, D), BF16)    # stats (bf16 ok)
    node_cols0 = din("node_cols0", (128, 4 * D), BF16)    # b0 tiles, [r,(t,d)]
    node_cols1 = din("node_cols1", (128, 4 * D), BF16)    # b1 tiles
    nodeT = din("nodeT", (D, B * L), BF16)                # transposed, proj
    ndTs_pack = din("ndTs_pack", (128, 256), BF16)        # shard T, [(r),(dc,i)]
    masks = din("masks", (128, 9))                        # col0 shard, 1+t full
    wlq = din("wlq", (128, 1024), BF16)                   # 8 blocks (dc,q)
    wr4_pack = din("wr4_pack", (128, 256), BF16)          # [(d%128),(dc,qc)]
    consts_row = din("consts_row", (1, 1152), BF16)       # s_r4|s_lq|ones_q
    w24bl4 = din("w24bl4", (128, 129))                    # w2_4 | b_l4

    # out[b, jc, sp, h, j, q, g, p]; i_in_core = b*64 + sp*32 + (h*4+g)*4 + q
    out = nc.dram_tensor("out", [B, 4, 2, 2, 128, 4 * 4 * PAIR], BF16,
                         kind="ExternalOutput").ap()

    with tile.TileContext(nc) as tc:
        with (
            tc.tile_pool(name="singles", bufs=1) as singles,
            tc.tile_pool(name="xrows", bufs=3) as xrows,
            tc.tile_pool(name="stats", bufs=4) as stats,
            tc.tile_pool(name="persist", bufs=1) as persist,
            tc.tile_pool(name="mp", bufs=3) as mp_pool,
            tc.tile_pool(name="stag", bufs=12) as stag_pool,
            tc.tile_pool(name="ps", bufs=4, space="PSUM") as ps,
        ):
            ident = singles.tile([128, 128], F32, tag="ident")
            make_identity(nc, ident)
            eps_t = singles.tile([128, 1], F32, tag="eps")
            nc.vector.memset(eps_t, LN_EPS)

            # ---------- loads ----------
            xfb = [xrows.tile([128, 4 * D], BF16, tag="xf", name=f"xfb{b}")
                   for b in range(B)]
            nc.sync.dma_start(out=xfb[0][:, 0:2 * D],
                              in_=node_cols0[:, 0:2 * D])
            msk = singles.tile([128, 9], F32, tag="msk")
            nc.sync.dma_start(out=msk, in_=masks[:, :])
            # tiles 2-3 on the gpsimd queue so their stat chains don't wait
            # behind the sync queue's ndT loads (head-of-line on DVE)
            nc.gpsimd.dma_start(out=xfb[0][:, 2 * D:4 * D],
                                in_=node_cols0[:, 2 * D:4 * D])
            ndT = [[persist.tile([128, L], BF16, tag=f"ndT{b}_{dc}",
                                 name=f"ndT{b}_{dc}") for dc in range(2)]
                   for b in range(B)]
            for dc in range(2):
                nc.sync.dma_start(out=ndT[0][dc],
                                  in_=nodeT[dc * 128:(dc + 1) * 128, 0:L])
            ndTs = singles.tile([128, 256], BF16, tag="ndTs")
            nc.sync.dma_start(out=ndTs, in_=ndTs_pack[:, :])

            xsr = xrows.tile([128, D], BF16, tag="xs", name="xsr")
            nc.scalar.dma_start(out=xsr, in_=node_cols_sh[:, :])
            wlq_sb = singles.tile([128, 1024], BF16, tag="wlq")
            nc.scalar.dma_start(out=wlq_sb, in_=wlq[:, :])
            crow = singles.tile([1, 1152], BF16, tag="crow")
            nc.scalar.dma_start(out=crow, in_=consts_row[:, :])
            w24bl = singles.tile([128, 129], F32, tag="w24bl")
            nc.scalar.dma_start(out=w24bl, in_=w24bl4[:, :])
            wr4_sb = singles.tile([128, 256], BF16, tag="wr4")
            nc.scalar.dma_start(out=wr4_sb, in_=wr4_pack[:, :])

            nc.gpsimd.dma_start(out=xfb[1], in_=node_cols1[:, :])
            for dc in range(2):
                nc.gpsimd.dma_start(out=ndT[1][dc],
                                    in_=nodeT[dc * 128:(dc + 1) * 128, L:2 * L])

            sr4_sb = crow[0:1, 0:128]
            slq_sb = crow[0:1, 128:640]
            onq_sb = crow[0:1, 640:1152]
            w24_sb = w24bl[:, 0:128]
            bl4_sb = w24bl[:, 128:129]

            # ---------- stat chain helper ----------
            def stat_chain(x_ap, mask_ap, rsm_dst):
                st = stats.tile([128, 6], F32, tag="st")
                nc.vector.bn_stats(out=st, in_=x_ap)
                mv = stats.tile([128, 2], F32, tag="mv")
                nc.vector.bn_aggr(out=mv, in_=st)
                sd = stats.tile([128, 1], F32, tag="sd")
                nc.scalar.activation(out=sd, in_=mv[:, 1:2],
                                     func=mybir.ActivationFunctionType.Sqrt,
                                     bias=eps_t, scale=1.0)
                rs = stats.tile([128, 1], F32, tag="rs")
                nc.vector.reciprocal(out=rs, in_=sd)
                nc.scalar.activation(out=rsm_dst, in_=rs,
                                     func=mybir.ActivationFunctionType.Copy,
                                     scale=mask_ap)
                return mv

            # ---------- shard path -> leftT_st [(q,c), 32 groups] ----------
            # single PSUM slot: psU [:,0:32] | psBc [:,256:288] | rows 512:768
            rsm_sh = singles.tile([128, 1], F32, tag="rsmsh")
            negmu_sh = singles.tile([1, 128], BF16, tag="negmush")
            rsmrow_sh = singles.tile([1, 128], BF16, tag="rsmrowsh")
            U_lst = singles.tile([128, 32], F32, tag="ulst")
            leftT_st = persist.tile([128, 32], F32, tag="leftst")
            sh_state = {}

            def shard_stats():
                mv_sh = stat_chain(xsr, msk[:, 0:1], rsm_sh)
                psh = ps.tile([128, 1024], F32, tag="ps", name="psh")
                nc.tensor.transpose(psh[0:1, 512:640], mv_sh[:, 0:1], ident)
                nc.tensor.transpose(psh[0:1, 640:768], rsm_sh, ident)
                nc.scalar.activation(out=negmu_sh, in_=psh[0:1, 512:640],
                                     func=mybir.ActivationFunctionType.Copy,
                                     scale=-1.0)
                nc.scalar.copy(out=rsmrow_sh, in_=psh[0:1, 640:768])
                sh_state["psh"] = psh

            def strided4(ap_row, q):
                r = ap_row.rearrange("r (g q) -> r q g", q=4)
                return r[:, q:q + 1, :]

            def build_left():
                psh = sh_state["psh"]
                psU = psh[:, 0:32]
                for dc in range(2):
                    nd3 = ndTs[:, dc * 128:(dc + 1) * 128].rearrange(
                        "d (g q) -> d q g", q=4)
                    for q in range(4):
                        nc.tensor.matmul(psU,
                                         wlq_sb[:, (dc * 4 + q) * 128:
                                                (dc * 4 + q + 1) * 128],
                                         nd3[:, q:q + 1, :],
                                         start=(dc == 0 and q == 0),
                                         stop=False)
                for q in range(4):
                    nc.tensor.matmul(psU, slq_sb[0:1, q * 128:(q + 1) * 128],
                                     strided4(negmu_sh, q),
                                     start=False, stop=(q == 3))
                nc.vector.tensor_copy(out=U_lst, in_=psU)
                psBc = psh[:, 256:288]
                for q in range(4):
                    nc.tensor.matmul(psBc, onq_sb[0:1, q * 128:(q + 1) * 128],
                                     strided4(rsmrow_sh, q),
                                     start=(q == 0), stop=(q == 3))
                nc.vector.tensor_mul(out=leftT_st, in0=U_lst, in1=psBc)
                nc.vector.tensor_scalar_add(out=leftT_st, in0=leftT_st,
                                            scalar1=bl4_sb)

            # ---------- right path per b -> U_r4[b] [(q,c), 512] ----------
            rsmc = persist.tile([128, 8], F32, tag="rsmc")   # col t = b*4+jc
            U_r4 = [persist.tile([128, L], BF16, tag=f"ur4{b}", name=f"ur4{b}")
                    for b in range(B)]

            def build_right(b):
                # per-jc slices: the first store depends on ONE stat chain,
                # not all four.  single PSUM slot: psUr [:,0:512] | mu rows
                pb_t = ps.tile([128, 1024], F32, tag="ps", name=f"pbr{b}")
                negmu = singles.tile([1, 512], BF16, tag=f"negmu{b}",
                                     name=f"negmu{b}")
                psUr = pb_t[:, 0:512]
                for t in range(4):
                    tt = b * 4 + t
                    mv = stat_chain(xfb[b][:, t * D:(t + 1) * D],
                                    msk[:, 1 + tt:2 + tt], rsmc[:, tt:tt + 1])
                    tsl = slice(t * 128, (t + 1) * 128)
                    nc.tensor.transpose(pb_t[0:1, 512 + t * 128:
                                             512 + (t + 1) * 128],
                                        mv[:, 0:1], ident)
                    nc.scalar.activation(out=negmu[0:1, tsl],
                                         in_=pb_t[0:1, 512 + t * 128:
                                                  512 + (t + 1) * 128],
                                         func=mybir.ActivationFunctionType.Copy,
                                         scale=-1.0)
                    nc.tensor.matmul(psUr[:, tsl], sr4_sb, negmu[0:1, tsl],
                                     start=True, stop=False)
                    nc.tensor.matmul(psUr[:, tsl], wr4_sb[:, 0:128],
                                     ndT[b][0][:, tsl],
                                     start=False, stop=False)
                    nc.tensor.matmul(psUr[:, tsl], wr4_sb[:, 128:256],
                                     ndT[b][1][:, tsl],
                                     start=False, stop=True)
                    if t % 2 == 0:
                        nc.vector.tensor_copy(out=U_r4[b][:, tsl],
                                              in_=psUr[:, tsl])
                    else:
                        nc.scalar.copy(out=U_r4[b][:, tsl], in_=psUr[:, tsl])

            # ---------- main loop ----------
            copy_cnt = [0]

            def scaled_copy(dst, src, scale_ap):
                on_dve = (copy_cnt[0] * 7) % 16 < 7
                copy_cnt[0] += 1
                if on_dve:
                    nc.vector.tensor_scalar_mul(out=dst, in0=src,
                                                scalar1=scale_ap)
                else:
                    nc.scalar.activation(out=dst, in_=src,
                                         func=mybir.ActivationFunctionType.Copy,
                                         scale=scale_ap)

            def main_block(b, sp):
                mp_all = mp_pool.tile([128, 1024], BF16, tag="mp",
                                      name=f"mp{b}_{sp}")
                for g in range(8):
                    col = b * 16 + sp * 8 + g
                    osl = mp_all[:, g * 128:(g + 1) * 128]
                    if b == 0 and sp == 0 and g % 2 == 1:
                        # first block: build on both engines to cut the head
                        nc.scalar.activation(
                            out=osl, in_=w24_sb,
                            func=mybir.ActivationFunctionType.Copy,
                            scale=leftT_st[:, col:col + 1])
                    else:
                        nc.vector.tensor_scalar_mul(
                            out=osl, in0=w24_sb,
                            scalar1=leftT_st[:, col:col + 1])
                for jc in range(4):
                    tt = b * 4 + jc
                    stg = stag_pool.tile([128, 4096], BF16, tag="stag",
                                         name=f"stg{b}{sp}{jc}")
                    for h in range(2):
                        pt = [ps.tile([128, 1024], F32, tag="ps",
                                      name=f"pt{b}{sp}{jc}{h}{k}")
                              for k in range(2)]
                        for q in range(4):
                            nc.tensor.matmul(
                                pt[q // 2][:, (q % 2) * 512:
                                           (q % 2 + 1) * 512],
                                U_r4[b][32 * q:32 * q + 32,
                                        jc * 128:(jc + 1) * 128],
                                mp_all[32 * q:32 * q + 32,
                                       h * 512:(h + 1) * 512],
                                start=True, stop=True,
                                tile_position=(32 * q, 0))
                        for k in range(2):
                            scaled_copy(stg[:, h * 2048 + k * 1024:
                                            h * 2048 + (k + 1) * 1024],
                                        pt[k], rsmc[:, tt:tt + 1])
                        nc.sync.dma_start(
                            out=out[b, jc, sp, h, :, :],
                            in_=stg[:, h * 2048:(h + 1) * 2048])

            shard_stats()
            build_left()
            build_right(0)
            main_block(0, 0)
            build_right(1)
            main_block(0, 1)
            main_block(1, 0)
            main_block(1, 1)

    nc.compile()
    names = ["node_cols_sh", "node_cols0", "node_cols1", "nodeT", "ndTs_pack",
             "masks", "wlq", "wr4_pack", "consts_row", "w24bl4"]
    return nc, names


def _prep_fast(node, mask, ln_gamma, ln_beta, W_left, b_left, W_right,
               b_right, W_out, b_out):
    import ml_dtypes
    f = np.float32
    bf = ml_dtypes.bfloat16
    node = np.ascontiguousarray(np.asarray(node, dtype=f))        # [B, L, D]
    mask_f = np.asarray(mask).astype(f)
    gamma = np.asarray(ln_gamma, dtype=f)
    W_l = np.asarray(W_left, dtype=f)
    W_r = np.asarray(W_right, dtype=f)
    b_l = np.asarray(b_left, dtype=f)
    W_o = np.asarray(W_out, dtype=f)

    s = 1.0 / np.sqrt(np.float32(DH))
    W_le = gamma[:, None] * W_l                    # [256, 32]
    W_re = gamma[:, None] * W_r * s
    s_l = W_le.sum(0)
    s_r = W_re.sum(0)

    wlq = np.zeros((8, 128, 128), f)
    for dc in range(2):
        for q in range(4):
            wlq[dc * 4 + q, :, 32 * q:32 * q + 32] = \
                W_le[dc * 128:(dc + 1) * 128, :]
    crow = np.zeros((1, 1152), f)
    crow[0, 0:128] = np.tile(s_r, 4)
    for q in range(4):
        crow[0, 128 + q * 128 + 32 * q:128 + q * 128 + 32 * q + 32] = s_l
        crow[0, 640 + q * 128 + 32 * q:640 + q * 128 + 32 * q + 32] = 1.0

    node_flat = node.reshape(B * L, D)
    mcol = np.zeros((128, 9), f)
    mcol[:, 1:9] = mask_f.reshape(-1, 128).T
    w24bl4 = np.empty((128, 129), f)
    w24bl4[:, 0:128] = np.tile(np.repeat(W_o, 2, axis=0), (4, 1))
    w24bl4[:, 128] = np.tile(b_l, 4)
    common = {
        "node_cols0": np.ascontiguousarray(
            node_flat[0:512].reshape(4, 128, D).transpose(1, 0, 2)
            .reshape(128, 4 * D)).astype(bf),
        "node_cols1": np.ascontiguousarray(
            node_flat[512:1024].reshape(4, 128, D).transpose(1, 0, 2)
            .reshape(128, 4 * D)).astype(bf),
        "nodeT": np.ascontiguousarray(node_flat.T).astype(bf),
        "wlq": np.ascontiguousarray(
            wlq.transpose(1, 0, 2).reshape(128, 1024)).astype(bf),
        "wr4_pack": np.ascontiguousarray(
            np.tile(W_re, (1, 4)).reshape(2, 128, 128).transpose(1, 0, 2)
            .reshape(128, 256)).astype(bf),
        "consts_row": crow.astype(bf),
        "w24bl4": w24bl4,
    }

    in_maps = []
    for c in range(NCORES):
        sl = slice(c * LSH, (c + 1) * LSH)
        shard = np.ascontiguousarray(node[:, sl, :].reshape(B * LSH, D))
        msk = mask_f[:, sl]
        m = dict(common)
        m["node_cols_sh"] = shard.astype(bf)
        m["ndTs_pack"] = np.ascontiguousarray(
            shard.T.reshape(2, 128, 128).transpose(1, 0, 2)
            .reshape(128, 256)).astype(bf)
        mc = mcol.copy()
        mc[:, 0] = msk.reshape(-1)
        m["masks"] = mc
        in_maps.append(m)
    return in_maps


def _unpack_fast(res):
    full = np.empty((B, L, L, PAIR), np.float32)
    for c in range(NCORES):
        dev = np.asarray(res.results[c]["out"]).reshape(
            B, 4, 2, 2, 128, 4, 4, PAIR)             # [b,jc,sp,h,j,q,g,p]
        full[:, c * LSH:(c + 1) * LSH] = (
            dev.transpose(0, 2, 3, 6, 5, 1, 4, 7)      # b,sp,h,g,q,jc,j,p
               .reshape(B, LSH, L, PAIR).astype(np.float32))
    return full


# ================================================================== general
def _build_general():
    nc = bacc.Bacc("TRN2", target_bir_lowering=False, debug=False,
                   num_devices=NCORES)

    def din(name, shape):
        return nc.dram_tensor(name, list(shape), F32, kind="ExternalInput").ap()

    node_full = din("node_full", (B * L, D))
    node_shard = din("node_shard", (B * LSH, D))
    mask_col_full = din("mask_col_full", (128, B * L // 128))
    mask_col_shard = din("mask_col_shard", (128, 1))
    mask_row_full = din("mask_row_full", (B, L))
    mask_row_shard = din("mask_row_shard", (1, B * LSH))
    w_left_e = din("w_left_e", (D + 1, DH))
    w_right_e = din("w_right_e", (D + 1, DH))
    b_left_col = din("b_left_col", (DH, 1))
    b_right_col = din("b_right_col", (DH, 1))
    w2 = din("w2", (DH + 1, PAIR))

    out = nc.dram_tensor("out", [B, 4, 4, 128, 16, PAIR], BF16,
                         kind="ExternalOutput").ap()

    NT_FULL = B * L // 128

    with tile.TileContext(nc) as tc:
        with (
            tc.tile_pool(name="singles", bufs=1) as singles,
            tc.tile_pool(name="xpool", bufs=9) as xpool,
            tc.tile_pool(name="stats", bufs=4) as stats,
            tc.tile_pool(name="persist", bufs=1) as persist,
            tc.tile_pool(name="mp", bufs=8) as mp_pool,
            tc.tile_pool(name="stag", bufs=12) as stag_pool,
            tc.tile_pool(name="ps_tp", bufs=1, space="PSUM") as ps_tp,
            tc.tile_pool(name="ps_proj", bufs=1, space="PSUM") as ps_proj,
            tc.tile_pool(name="ps_big", bufs=5, space="PSUM") as ps_big,
        ):
            ident = singles.tile([128, 128], F32, tag="ident")
            make_identity(nc, ident)
            eps_t = singles.tile([128, 1], F32, tag="eps")
            nc.vector.memset(eps_t, LN_EPS)

            xs = xpool.tile([128, D], F32, tag="x", name="xs")
            nc.sync.dma_start(out=xs, in_=node_shard[:, :])
            mcs_sb = singles.tile([128, 1], F32, tag="mcs")
            nc.sync.dma_start(out=mcs_sb, in_=mask_col_shard[:, :])
            xf_tiles = [None] * NT_FULL
            for t in range(NT_FULL):
                xf = xpool.tile([128, D], F32, tag="x", name=f"xf{t}")
                nc.sync.dma_start(out=xf,
                                  in_=node_full[t * 128:(t + 1) * 128, :])
                xf_tiles[t] = xf
            mcf_sb = singles.tile([128, NT_FULL], F32, tag="mcf")
            nc.sync.dma_start(out=mcf_sb, in_=mask_col_full[:, :])
            wl_sb = [singles.tile([128, DH], F32, tag=f"wl{dc}", name=f"wl{dc}")
                     for dc in range(2)]
            wl_row = singles.tile([1, DH], F32, tag="wlrow")
            for dc in range(2):
                nc.sync.dma_start(out=wl_sb[dc],
                                  in_=w_left_e[dc * 128:(dc + 1) * 128, :])
            nc.sync.dma_start(out=wl_row, in_=w_left_e[D:D + 1, :])
            bl_sb = singles.tile([DH, 1], F32, tag="bl")
            nc.sync.dma_start(out=bl_sb, in_=b_left_col[:, :])
            w2_sb = singles.tile([DH + 1, PAIR], F32, tag="w2")
            nc.sync.dma_start(out=w2_sb, in_=w2[:, :])

            wr_sb = [singles.tile([128, DH], F32, tag=f"wr{dc}", name=f"wr{dc}")
                     for dc in range(2)]
            wr_row = singles.tile([1, DH], F32, tag="wrrow")
            for dc in range(2):
                nc.gpsimd.dma_start(out=wr_sb[dc],
                                    in_=w_right_e[dc * 128:(dc + 1) * 128, :])
            nc.gpsimd.dma_start(out=wr_row, in_=w_right_e[D:D + 1, :])
            br_sb = singles.tile([DH, 1], F32, tag="br")
            nc.gpsimd.dma_start(out=br_sb, in_=b_right_col[:, :])
            mrf_sb = [singles.tile([1, L], F32, tag=f"mrf{b}", name=f"mrf{b}")
                      for b in range(B)]
            for b in range(B):
                nc.gpsimd.dma_start(out=mrf_sb[b],
                                    in_=mask_row_full[b:b + 1, :])
            mrs_sb = singles.tile([1, B * LSH], F32, tag="mrs")
            nc.gpsimd.dma_start(out=mrs_sb, in_=mask_row_shard[:, :])
            ones_row = singles.tile([1, L], BF16, tag="ones")
            nc.vector.memset(ones_row, 1.0)

            def layernorm_masked(x_t, mask_col_ap):
                st = stats.tile([128, 6], F32, tag="st")
                nc.vector.bn_stats(out=st, in_=x_t)
                mv = stats.tile([128, 2], F32, tag="mv")
                nc.vector.bn_aggr(out=mv, in_=st)
                sd = stats.tile([128, 1], F32, tag="sd")
                nc.scalar.activation(out=sd, in_=mv[:, 1:2],
                                     func=mybir.ActivationFunctionType.Sqrt,
                                     bias=eps_t, scale=1.0)
                rs = stats.tile([128, 1], F32, tag="rs")
                nc.vector.reciprocal(out=rs, in_=sd)
                rsm = stats.tile([128, 1], F32, tag="rsm")
                nc.vector.tensor_mul(out=rsm, in0=rs, in1=mask_col_ap)
                nc.vector.tensor_scalar(out=x_t, in0=x_t,
                                        scalar1=mv[:, 0:1], scalar2=rsm,
                                        op0=mybir.AluOpType.subtract,
                                        op1=mybir.AluOpType.mult)

            layernorm_masked(xs, mcs_sb[:, 0:1])
            xsT = [persist.tile([128, B * LSH], F32, tag=f"xsT{dc}",
                                name=f"xsT{dc}") for dc in range(2)]
            for dc in range(2):
                pt = ps_tp.tile([128, 128], F32, tag="tp")
                nc.tensor.transpose(pt, xs[:, dc * 128:(dc + 1) * 128], ident)
                nc.scalar.copy(out=xsT[dc], in_=pt)

            ps_l = ps_proj.tile([DH, L], F32, tag="pr", name="ps_l")
            ps_l = ps_l[:, 0:B * LSH]
            for dc in range(2):
                nc.tensor.matmul(ps_l, wl_sb[dc], xsT[dc],
                                 start=(dc == 0), stop=False)
            nc.tensor.matmul(ps_l, wl_row, mrs_sb, start=False, stop=True)
            leftT = persist.tile([DH + 1, B * LSH], F32, tag="leftT")
            nc.vector.tensor_scalar_add(out=leftT[0:DH, :], in0=ps_l,
                                        scalar1=bl_sb)
            nc.vector.memset(leftT[DH:DH + 1, :], 1.0)

            rightT = [persist.tile([DH + 1, L], BF16, tag=f"rt{b}",
                                   name=f"rt{b}") for b in range(B)]
            xT = [[persist.tile([128, L], F32, tag=f"xT{b}_{dc}",
                                name=f"xT{b}_{dc}") for dc in range(2)]
                  for b in range(B)]
            for b in range(B):
                for lc in range(4):
                    t = b * 4 + lc
                    xft = xf_tiles[t]
                    layernorm_masked(xft, mcf_sb[:, t:t + 1])
                    for dc in range(2):
                        pt = ps_tp.tile([128, 128], F32, tag="tp")
                        nc.tensor.transpose(pt, xft[:, dc * 128:(dc + 1) * 128],
                                            ident)
                        if (lc + dc) % 2 == 0:
                            nc.vector.tensor_copy(
                                out=xT[b][dc][:, lc * 128:(lc + 1) * 128],
                                in_=pt)
                        else:
                            nc.scalar.copy(
                                out=xT[b][dc][:, lc * 128:(lc + 1) * 128],
                                in_=pt)

                ps_r = ps_proj.tile([DH, L], F32, tag="pr")
                for jc in range(4):
                    jsl = slice(jc * 128, (jc + 1) * 128)
                    for dc in range(2):
                        nc.tensor.matmul(ps_r[:, jsl], wr_sb[dc],
                                         xT[b][dc][:, jsl],
                                         start=(dc == 0), stop=False)
                    nc.tensor.matmul(ps_r[:, jsl], wr_row, mrf_sb[b][:, jsl],
                                     start=False, stop=True)
                    nc.vector.tensor_scalar_add(out=rightT[b][0:DH, jsl],
                                                in0=ps_r[:, jsl],
                                                scalar1=br_sb)
                nc.vector.tensor_copy(out=rightT[b][DH:DH + 1, :],
                                      in_=ones_row)

            copy_cnt = [0]

            def psum_copy(dst, src):
                on_dve = (copy_cnt[0] * 11) % 32 < 11
                copy_cnt[0] += 1
                if on_dve:
                    nc.vector.tensor_copy(out=dst, in_=src)
                else:
                    nc.scalar.copy(out=dst, in_=src)

            def build_mps(b, sg):
                mps = []
                for il in range(4):
                    mp = mp_pool.tile([DH + 1, 512], BF16, tag="mp",
                                      name=f"mp{b}_{sg}_{il}")
                    for q in range(4):
                        col = b * LSH + (sg * 4 + il) * 4 + q
                        nc.vector.tensor_scalar_mul(
                            out=mp[:, q * 128:(q + 1) * 128], in0=w2_sb,
                            scalar1=leftT[:, col:col + 1])
                    mps.append(mp)
                return mps

            for b in range(B):
                for sg in range(4):
                    mps = build_mps(b, sg)
                    for jc in range(4):
                        lhsT = rightT[b][:, jc * 128:(jc + 1) * 128]
                        stg = stag_pool.tile([128, 16 * 128], BF16, tag="stag")
                        for il in range(4):
                            pb = ps_big.tile([128, 512], F32, tag="big")
                            nc.tensor.matmul(pb, lhsT, mps[il],
                                             start=True, stop=True)
                            psum_copy(stg[:, il * 512:(il + 1) * 512], pb)
                        dst_ap = out[b, jc, sg, :, :, :]
                        src_ap = stg[:, :].rearrange("j (i p) -> j i p", p=128)
                        deng = nc.sync if jc % 2 == 0 else nc.gpsimd
                        deng.dma_start(out=dst_ap, in_=src_ap)

    nc.compile()
    names = ["node_full", "node_shard", "mask_col_full", "mask_col_shard",
             "mask_row_full", "mask_row_shard", "w_left_e", "w_right_e",
             "b_left_col", "b_right_col", "w2"]
    return nc, names


def _prep_general(node, mask, ln_gamma, ln_beta, W_left, b_left, W_right,
                  b_right, W_out, b_out):
    f = np.float32
    node = np.ascontiguousarray(np.asarray(node, dtype=f))
    mask_f = np.asarray(mask).astype(f)
    gamma = np.asarray(ln_gamma, dtype=f)
    beta = np.asarray(ln_beta, dtype=f)
    W_l = np.asarray(W_left, dtype=f)
    W_r = np.asarray(W_right, dtype=f)
    b_l = np.asarray(b_left, dtype=f)
    b_r = np.asarray(b_right, dtype=f)
    W_o = np.asarray(W_out, dtype=f)
    b_o = np.asarray(b_out, dtype=f)

    s = 1.0 / np.sqrt(np.float32(DH))
    w_left_e = np.concatenate([gamma[:, None] * W_l, (beta @ W_l)[None, :]], 0)
    w_right_e = np.concatenate([gamma[:, None] * W_r, (beta @ W_r)[None, :]],
                               0) * s
    w2 = np.concatenate([np.repeat(W_o, 2, axis=0), b_o[None, :]], 0)

    node_flat = node.reshape(B * L, D)
    common = {
        "node_full": node_flat,
        "mask_col_full": np.ascontiguousarray(mask_f.reshape(-1, 128).T),
        "mask_row_full": np.ascontiguousarray(mask_f),
        "w_left_e": np.ascontiguousarray(w_left_e),
        "w_right_e": np.ascontiguousarray(w_right_e),
        "b_left_col": np.ascontiguousarray(b_l[:, None]),
        "b_right_col": np.ascontiguousarray(b_r[:, None] * s),
        "w2": np.ascontiguousarray(w2),
    }

    in_maps = []
    for c in range(NCORES):
        sl = slice(c * LSH, (c + 1) * LSH)
        shard = np.ascontiguousarray(node[:, sl, :].reshape(B * LSH, D))
        msk = mask_f[:, sl]
        m = dict(common)
        m["node_shard"] = shard
        m["mask_col_shard"] = np.ascontiguousarray(msk.reshape(-1)[:, None])
        m["mask_row_shard"] = np.ascontiguousarray(msk.reshape(1, -1))
        in_maps.append(m)
    return in_maps


def _unpack_general(res):
    full = np.empty((B, L, L, PAIR), np.float32)
    for c in range(NCORES):
        dev = np.asarray(res.results[c]["out"])  # [b, jc, sg, j, i16, p]
        full[:, c * LSH:(c + 1) * LSH] = (
            dev.transpose(0, 2, 4, 1, 3, 5)
               .reshape(B, LSH, L, PAIR).astype(np.float32))
    return full


# ================================================================ dispatch
def _is_fast(inputs):
    z = lambda k: not np.any(np.asarray(inputs[k]))
    return z("ln_beta") and z("b_right") and z("b_out")


def _get_program(fast):
    global _COMPILED
    key = "fast" if fast else "general"
    if key not in _CACHE:
        _CACHE[key] = _build_fast() if fast else _build_general()
    _COMPILED = _CACHE[key]
    return _CACHE[key]


def _prepare_in_maps(**inputs):
    fast = _is_fast(inputs)
    _get_program(fast)
    return (_prep_fast if fast else _prep_general)(**inputs)


def kernel(**inputs):
    fast = _is_fast(inputs)
    nc, names = _get_program(fast)
    in_maps = (_prep_fast if fast else _prep_general)(**inputs)
    res = run_bass_kernel_spmd(nc, in_maps, core_ids=list(range(NCORES)))
    return (_unpack_fast if fast else _unpack_general)(res)


if __name__ == "__main__":
    rng = np.random.default_rng(1)

    def np_reference(node, mask, ln_gamma, ln_beta, W_left, b_left, W_right,
                     b_right, W_out, b_out):
        node = node.astype(np.float64)
        mu = node.mean(-1, keepdims=True)
        var = ((node - mu) ** 2).mean(-1, keepdims=True)
        x = (node - mu) / np.sqrt(var + LN_EPS) * ln_gamma + ln_beta
        x = x * mask[..., None]
        left = (x @ W_left + b_left).reshape(B, L, H, -1)
        right = ((x @ W_right + b_right) / np.sqrt(DH)).reshape(B, L, H, -1)
        o = np.einsum("bihk,bjhk->bijh", left, right)
        return np.einsum("bijh,hp->bijp", o, W_out) + b_out

    # --- fast path (benchmark-like: beta/b_r/b_out zero, mask+gamma general)
    mask = np.ones((B, L), dtype=bool)
    mask[0, 500:] = False
    mask[1, :3] = False
    fast_inputs = {
        "node": rng.standard_normal((B, L, D)).astype(np.float32),
        "mask": mask,
        "ln_gamma": (1.0 + 0.1 * rng.standard_normal(D)).astype(np.float32),
        "ln_beta": np.zeros(D, np.float32),
        "W_left": (rng.standard_normal((D, DH)) / np.sqrt(D)).astype(np.float32),
        "b_left": (0.1 * rng.standard_normal(DH)).astype(np.float32),
        "W_right": (rng.standard_normal((D, DH)) / np.sqrt(D)).astype(np.float32),
        "b_right": np.zeros(DH, np.float32),
        "W_out": (rng.standard_normal((H, PAIR)) / np.sqrt(H)).astype(np.float32),
        "b_out": np.zeros(PAIR, np.float32),
    }
    got = kernel(**fast_inputs)
    exp = np_reference(**fast_inputs)
    rel = np.abs(got - exp).max() / np.abs(exp).max()
    print("fast-path rel err:", rel)
    assert rel < 1.8e-2, rel

    # --- general path (everything nonzero)
    gen_inputs = dict(fast_inputs)
    gen_inputs["ln_beta"] = (0.1 * rng.standard_normal(D)).astype(np.float32)
    gen_inputs["b_right"] = (0.1 * rng.standard_normal(DH)).astype(np.float32)
    gen_inputs["b_out"] = (0.1 * rng.standard_normal(PAIR)).astype(np.float32)
    got = kernel(**gen_inputs)
    exp = np_reference(**gen_inputs)
    rel = np.abs(got - exp).max() / np.abs(exp).max()
    print("general-path rel err:", rel)
    assert rel < 1.8e-2, rel
    print("OK")
